# revision 28
# baseline (speedup 1.0000x reference)
"""Causal GQA self-attention (B=2, S=2048, D=2048, 32 Q heads / 8 KV heads,
hd=64, RoPE) on 8 TRN2 NeuronCores.

Sharding: 2-way data parallel over batch x 4-way tensor parallel over heads.
Core c handles batch b=c//4 and head group g=c%4 (8 Q heads, 2 KV heads).
Attention outputs (transposed, feature-major) are AllGathered within each
4-core group per 512-token block; each core then computes a 512-column slice
of the final out-projection for that block, overlapping the collective and
out-projection with the next block's attention. Host reassembles the full
[2,2048,2048] output.

Matmuls run bf16 x bf16 -> fp32 PSUM; softmax/normalization in fp32.
Elementwise work is split across the vector (PSUM-reading ops), gpsimd
(SBUF-only ops), and scalar (PSUM->SBUF copies, exp) engines so no single
engine serializes the tensor engine.
"""
import sys
sys.path.insert(0, "/opt/trn_rl_repo")
import numpy as np
import ml_dtypes
import concourse.bass as bass
import concourse.mybir as mybir
import concourse.tile as tile
from concourse import bacc
from concourse.bass_utils import run_bass_kernel_spmd
from concourse.masks import make_identity

MODEL_DIM = 2048
SEQ = 2048
HEAD_DIM = 64
ROPE_BASE = 10000.0
BATCH = 2
NCORES = 8
GROUPS = [[0, 1, 2, 3], [4, 5, 6, 7]]
QF = 512   # q features per core (8 heads * 64)
KF = 128   # kv features per core (2 kv heads * 64)

f32 = mybir.dt.float32
bf16 = mybir.dt.bfloat16
ACTF = mybir.ActivationFunctionType
BF = ml_dtypes.bfloat16

_cache = {}


def _build_kernel():
    nc = bacc.Bacc(None, target_bir_lowering=False, debug=False,
                   num_devices=NCORES)
    xT = nc.dram_tensor("xT", [MODEL_DIM, SEQ], bf16, kind="ExternalInput").ap()
    wq = nc.dram_tensor("wq", [MODEL_DIM, QF], bf16, kind="ExternalInput").ap()
    wk = nc.dram_tensor("wk", [MODEL_DIM, KF], bf16, kind="ExternalInput").ap()
    wv = nc.dram_tensor("wv", [MODEL_DIM, KF], bf16, kind="ExternalInput").ap()
    wo = nc.dram_tensor("wo", [MODEL_DIM, QF], bf16, kind="ExternalInput").ap()
    p2 = nc.dram_tensor("p2", [128, 128], bf16, kind="ExternalInput").ap()
    cosr = nc.dram_tensor("cosr", [128, SEQ], f32, kind="ExternalInput").ap()
    sinr = nc.dram_tensor("sinr", [128, SEQ], f32, kind="ExternalInput").ap()
    masks = nc.dram_tensor("masks", [128, 4 * 512], bf16, kind="ExternalInput").ap()
    out = nc.dram_tensor("out", [SEQ, QF], f32, kind="ExternalOutput").ap()

    with tile.TileContext(nc) as tc:
        from contextlib import ExitStack
        with ExitStack() as ctx:
            persist = ctx.enter_context(tc.tile_pool(name="persist", bufs=1))
            consts = ctx.enter_context(tc.tile_pool(name="consts", bufs=1))
            dram = ctx.enter_context(tc.tile_pool(name="dram", bufs=1, space="DRAM"))

            qT = [persist.tile([128, SEQ], bf16, tag=f"qT{i}", name=f"qT{i}")
                  for i in range(4)]
            kT = persist.tile([128, SEQ], bf16, tag="kT")
            # partition-swapped copy (kv1 on 0:64, kv0 on 64:128): matmul
            # requires lhsT/rhs at the same base partition as the q slice
            kT2 = persist.tile([128, SEQ], bf16, tag="kT2")
            V1 = persist.tile([128, 16, 132], bf16, tag="V1")

            p2_sb = consts.tile([128, 128], bf16, tag="p2")
            ones_sb = consts.tile([1, 64], bf16, tag="ones")
            ident = consts.tile([128, 128], bf16, tag="ident")
            masks_sb = consts.tile([128, 4 * 512], bf16, tag="masks")
            wo_sb = consts.tile([128, 16, QF], bf16, tag="wo")

            nc.vector.memset(ones_sb[:], 1.0)
            make_identity(nc, ident[:])
            nc.vector.memset(V1[:, :, 64:65], 1.0)    # ones col for kv head 0
            nc.vector.memset(V1[:, :, 130:131], 1.0)  # ones col for kv head 1

            y_loc = [dram.tile([QF, 512], bf16, tag=f"yl{j}", name=f"yl{j}")
                     for j in range(4)]
            y_ful = [dram.tile([4 * QF, 512], bf16, tag=f"yf{j}", name=f"yf{j}")
                     for j in range(4)]
            # block 3 gathers in two halves (heads 0-3 / 4-7) so the final
            # gather overlaps the last heads' compute
            y_l3 = [dram.tile([QF // 2, 512], bf16, tag=f"yl3{i}",
                              name=f"yl3{i}") for i in range(2)]
            y_f3 = [dram.tile([2 * QF, 512], bf16, tag=f"yf3{i}",
                              name=f"yf3{i}") for i in range(2)]

            # ---------------- Phase 1: QKV projections + RoPE ----------------
            with tc.tile_pool(name="ph1w", bufs=1) as ph1w, \
                 tc.tile_pool(name="xts", bufs=3) as xts, \
                 tc.tile_pool(name="ph1t", bufs=3) as ph1t, \
                 tc.tile_pool(name="ph1ps", bufs=1, space="PSUM") as pp, \
                 tc.tile_pool(name="rotps", bufs=2, space="PSUM") as rp:
                wq_sb = ph1w.tile([128, 16, QF], bf16, tag="wq")
                wk_sb = ph1w.tile([128, 16, KF], bf16, tag="wk")
                wv_sb = ph1w.tile([128, 16, KF], bf16, tag="wv")
                cos_sb = ph1w.tile([128, SEQ], f32, tag="cos")
                sin_sb = ph1w.tile([128, SEQ], f32, tag="sin")
                vT = ph1w.tile([128, SEQ], bf16, tag="vT")
                # interleave sb=0 x-tiles with chunked wq so the PE starts
                # after ~1 MB of DMA instead of the full weight preload;
                # p2/masks/wo aren't needed until much later, so they go last
                wq_r = wq.rearrange("(c p) m -> p c m", p=128)
                xt0 = []
                for c4 in range(4):
                    for dc in range(4 * c4, 4 * c4 + 4):
                        xt = xts.tile([128, 512], bf16, tag="xt0",
                                      name="xt0", bufs=16)
                        nc.sync.dma_start(
                            xt[:], xT[dc * 128:(dc + 1) * 128, 0:512])
                        xt0.append(xt)
                    nc.sync.dma_start(wq_sb[:, 4 * c4:4 * c4 + 4, :],
                                      wq_r[:, 4 * c4:4 * c4 + 4, :])
                nc.sync.dma_start(wk_sb[:], wk.rearrange("(c p) m -> p c m", p=128))
                nc.sync.dma_start(wv_sb[:], wv.rearrange("(c p) m -> p c m", p=128))
                nc.sync.dma_start(cos_sb[:], cosr[:])
                nc.sync.dma_start(sin_sb[:], sinr[:])
                nc.sync.dma_start(p2_sb[:], p2[:])
                nc.sync.dma_start(masks_sb[:], masks[:])
                nc.sync.dma_start(wo_sb[:],
                                  wo.rearrange("(c p) n -> p c n", p=128))

                for sb in range(4):
                    ssl = slice(sb * 512, (sb + 1) * 512)
                    psums = [pp.tile([128, 512], f32, tag=f"acc{m}", name=f"acc{m}")
                             for m in range(6)]
                    if sb == 0:
                        # three passes (Q, K, V) so the PE starts on the first
                        # wq chunk instead of waiting for all weight DMAs
                        for dc in range(16):
                            st, sp = (dc == 0), (dc == 15)
                            for qi in range(4):
                                nc.tensor.matmul(
                                    psums[qi][:],
                                    wq_sb[:, dc, qi * 128:(qi + 1) * 128],
                                    xt0[dc][:], start=st, stop=sp)
                        for dc in range(16):
                            nc.tensor.matmul(psums[4][:], wk_sb[:, dc, :],
                                             xt0[dc][:], start=(dc == 0),
                                             stop=(dc == 15))
                        for dc in range(16):
                            nc.tensor.matmul(psums[5][:], wv_sb[:, dc, :],
                                             xt0[dc][:], start=(dc == 0),
                                             stop=(dc == 15))
                    else:
                        for dc in range(16):
                            xt = xts.tile([128, 512], bf16)
                            nc.sync.dma_start(
                                xt[:], xT[dc * 128:(dc + 1) * 128, ssl])
                            st, sp = (dc == 0), (dc == 15)
                            for qi in range(4):
                                nc.tensor.matmul(
                                    psums[qi][:],
                                    wq_sb[:, dc, qi * 128:(qi + 1) * 128],
                                    xt[:], start=st, stop=sp)
                            nc.tensor.matmul(psums[4][:], wk_sb[:, dc, :],
                                             xt[:], start=st, stop=sp)
                            nc.tensor.matmul(psums[5][:], wv_sb[:, dc, :],
                                             xt[:], start=st, stop=sp)
                    # RoPE: rope(z) = z*cos + (z @ P)*sin, P applied on the PE.
                    # One scalar-engine copy frees the psum; the cos-mul runs
                    # on gpsimd from the bf16 copy so the vector engine only
                    # touches the rotation psum.
                    for src, dst in [(psums[0], qT[0]), (psums[1], qT[1]),
                                     (psums[2], qT[2]), (psums[3], qT[3]),
                                     (psums[4], kT)]:
                        tq = ph1t.tile([128, 512], bf16, tag="tq")
                        nc.scalar.copy(tq[:], src[:])
                        pr = rp.tile([128, 512], f32, tag="rot")
                        nc.tensor.matmul(pr[:], p2_sb[:], tq[:])
                        t1 = ph1t.tile([128, 512], f32, tag="t1")
                        t2 = ph1t.tile([128, 512], f32, tag="t2")
                        nc.gpsimd.tensor_mul(t1[:], tq[:], cos_sb[:, ssl])
                        nc.vector.tensor_mul(t2[:], pr[:], sin_sb[:, ssl])
                        nc.vector.tensor_add(dst[:, ssl], t1[:], t2[:])
                    nc.scalar.copy(vT[:, ssl], psums[5][:])

                    # V: transpose [kvf, S] -> natural [S, kvf] per 128-chunk
                    for cl in range(4):
                        c = sb * 4 + cl
                        pt = rp.tile([128, 128], bf16, tag="rot", name="pt")
                        nc.tensor.transpose(pt[:], vT[:, c * 128:(c + 1) * 128],
                                            ident[:])
                        nc.vector.tensor_copy(V1[:, c, 0:64], pt[:, 0:64])
                        nc.vector.tensor_copy(V1[:, c, 66:130], pt[:, 64:128])

                    # partition-swapped kT copy for this seq chunk
                    nc.vector.tensor_copy(kT2[0:64, ssl], kT[64:128, ssl])
                    nc.vector.tensor_copy(kT2[64:128, ssl], kT[0:64, ssl])

            # -------- Phase 2+3: attention, per-block AllGather, out-proj ----
            with tc.tile_pool(name="sps", bufs=2, space="PSUM") as sp_pool, \
                 tc.tile_pool(name="avps", bufs=2, space="PSUM") as avp, \
                 tc.tile_pool(name="ubnk", bufs=2, space="PSUM") as ubnk, \
                 tc.tile_pool(name="expp", bufs=4) as ep, \
                 tc.tile_pool(name="p2t", bufs=4) as p2t, \
                 tc.tile_pool(name="ystg", bufs=2) as ystg, \
                 tc.tile_pool(name="yst", bufs=2) as yst, \
                 tc.tile_pool(name="otp", bufs=3) as otp:
                def load_ybig(j):
                    ybig = yst.tile([128, 16, 512], bf16, tag="ybig",
                                    name="ybig")
                    nc.sync.dma_start(
                        ybig[:], y_ful[j].rearrange("(c p) s -> p c s", p=128))
                    return ybig

                def outproj_s4(ybig, j, s4):
                    # 128-token slice of block j's out projection; interleaved
                    # between attention heads of block j+1 so the PE chews on
                    # it during the exp-bound stretches and never waits on
                    # the AllGather.
                    pso = ubnk.tile([128, 512], f32, tag="u", name="pso")
                    for fc in range(16):
                        nc.tensor.matmul(
                            pso[:], ybig[:, fc, s4 * 128:(s4 + 1) * 128],
                            wo_sb[:, fc, :], start=(fc == 0), stop=(fc == 15))
                    ot = otp.tile([128, 512], f32)
                    nc.vector.tensor_copy(ot[:], pso[:])
                    r0 = (j * 4 + s4) * 128
                    nc.sync.dma_start(out[r0:r0 + 128, :], ot[:])

                for j in range(4):
                    jsl = slice(j * 512, (j + 1) * 512)
                    ySB = ystg.tile([128, 4, 512], bf16, tag="ySB", name="ySB")
                    for h in range(8):
                        kv = h // 4
                        qt = qT[h // 2]
                        qp = 64 * (h % 2)
                        vcol = slice(0, 65) if kv == 0 else slice(66, 131)
                        ksrc = kT if qp == 64 * kv else kT2
                        kpart = slice(qp, qp + 64)
                        qap = qt[qp:qp + 64, jsl]
                        pav = avp.tile([65, 512], f32, tag="pav")
                        ni = 4 * j + 4
                        for ip in range(ni // 2):
                            ps = sp_pool.tile([128, 2, 512], f32, tag="ps")
                            for b in range(2):
                                i = 2 * ip + b
                                nc.tensor.matmul(
                                    ps[:, b, :],
                                    ksrc[kpart, i * 128:(i + 1) * 128],
                                    qap, start=True, stop=True)
                            et = ep.tile([128, 2, 512], bf16, tag="et")
                            nc.scalar.activation(et[:], ps[:], ACTF.Exp,
                                                 scale=0.125)
                            tp = 2 * ip - 4 * j  # t of first half
                            if tp >= 0:  # both halves in the diagonal band
                                nc.vector.tensor_mul(
                                    et[:], et[:],
                                    masks_sb[:, tp * 512:(tp + 2) * 512])
                            for b in range(2):
                                i = 2 * ip + b
                                nc.tensor.matmul(pav[:], V1[:, i, vcol],
                                                 et[:, b, :],
                                                 start=(i == 0), stop=(i == ni - 1))
                        # normalize by the fused denominator row (row 64)
                        rcp = p2t.tile([1, 512], bf16, tag="rcp")
                        with nc.allow_low_precision(reason="softmax denom bcast"):
                            nc.vector.reciprocal(rcp[0:1, :], pav[64:65, :])
                        pbc = ubnk.tile([128, 512], f32, tag="u", name="pbc")
                        nc.tensor.matmul(pbc[0:64, :], ones_sb[:], rcp[:])
                        bc = p2t.tile([64, 512], f32, tag="bc")
                        nc.vector.tensor_copy(bc[:], pbc[0:64, :])
                        yrow = slice(64 * (h % 2), 64 * (h % 2) + 64)
                        nc.vector.tensor_mul(ySB[yrow, h // 2, :],
                                             pav[0:64, :], bc[:])
                        if j == 3 and h in (3, 7):
                            hf = h // 4
                            nc.sync.dma_start(
                                y_l3[hf].rearrange(
                                    "(c q p) s -> (q p) c s", q=2, p=64),
                                ySB[:, 2 * hf:2 * hf + 2, :])
                            nc.gpsimd.collective_compute(
                                "AllGather", mybir.AluOpType.bypass,
                                ins=[y_l3[hf].opt()], outs=[y_f3[hf].opt()],
                                replica_groups=GROUPS)
                    if j < 3:
                        nc.sync.dma_start(
                            y_loc[j].rearrange(
                                "(c q p) s -> (q p) c s", q=2, p=64),
                            ySB[:])
                        nc.gpsimd.collective_compute(
                            "AllGather", mybir.AluOpType.bypass,
                            ins=[y_loc[j].opt()], outs=[y_ful[j].opt()],
                            replica_groups=GROUPS)
                    if j >= 1:
                        ybig_prev = load_ybig(j - 1)
                        for s4 in range(4):
                            outproj_s4(ybig_prev, j - 1, s4)
                yb3 = []
                for hf in range(2):
                    yb = yst.tile([128, 8, 512], bf16, tag="ybig",
                                  name="ybig")
                    nc.sync.dma_start(
                        yb[:], y_f3[hf].rearrange("(c p) s -> p c s", p=128))
                    yb3.append(yb)
                for s4 in range(4):
                    pso = ubnk.tile([128, 512], f32, tag="u", name="pso")
                    n = 0
                    for hf in range(2):
                        for ca in range(8):
                            fc = (ca // 2) * 4 + 2 * hf + (ca % 2)
                            nc.tensor.matmul(
                                pso[:], yb3[hf][:, ca, s4 * 128:(s4 + 1) * 128],
                                wo_sb[:, fc, :], start=(n == 0), stop=(n == 15))
                            n += 1
                    ot = otp.tile([128, 512], f32)
                    nc.vector.tensor_copy(ot[:], pso[:])
                    r0 = (3 * 4 + s4) * 128
                    nc.sync.dma_start(out[r0:r0 + 128, :], ot[:])

    nc.compile()
    return nc


def _host_constants():
    inv_freq = (1.0 / (ROPE_BASE ** (np.arange(0, HEAD_DIM, 2, dtype=np.float32)
                                     / HEAD_DIM))).astype(np.float32)
    t = np.arange(SEQ, dtype=np.float32)
    freqs = np.outer(t, inv_freq)                      # [S, 32]
    emb = np.concatenate([freqs, freqs], axis=-1)      # [S, 64]
    cosT = np.cos(emb).astype(np.float32).T            # [64, S]
    sinT = np.sin(emb).astype(np.float32).T
    cosr = np.ascontiguousarray(np.vstack([cosT, cosT]))   # [128, S]
    sinr = np.ascontiguousarray(np.vstack([sinT, sinT]))

    # rotation matrix: rot(z)[m] = -z[m+32] (m<32), z[m-32] (m>=32); 2 blocks
    R = np.zeros((64, 64), dtype=np.float32)
    for d in range(32):
        R[d + 32, d] = -1.0
        R[d, d + 32] = 1.0
    p2 = np.zeros((128, 128), dtype=np.float32)
    p2[0:64, 0:64] = R
    p2[64:128, 64:128] = R

    k_idx = np.arange(128)[:, None]
    q_idx = np.arange(512)[None, :]
    m = np.concatenate(
        [(128 * t_ + k_idx <= q_idx).astype(np.float32) for t_ in range(4)],
        axis=1)                                        # [128, 2048]
    return cosr, sinr, p2.astype(BF), np.ascontiguousarray(m).astype(BF)


def _in_maps(x, Wq, Wk, Wv, Wo):
    cosr, sinr, p2, masks = _host_constants()
    xb = [np.ascontiguousarray(x[b].T).astype(BF) for b in range(BATCH)]
    wqb = Wq.astype(BF)
    wkb = Wk.astype(BF)
    wvb = Wv.astype(BF)
    wob = Wo.astype(BF)
    maps = []
    for c in range(NCORES):
        b, g = c // 4, c % 4
        maps.append({
            "xT": xb[b],
            "wq": np.ascontiguousarray(wqb[:, g * QF:(g + 1) * QF]),
            "wk": np.ascontiguousarray(wkb[:, g * KF:(g + 1) * KF]),
            "wv": np.ascontiguousarray(wvb[:, g * KF:(g + 1) * KF]),
            "wo": np.ascontiguousarray(wob[:, g * QF:(g + 1) * QF]),
            "p2": p2, "cosr": cosr, "sinr": sinr, "masks": masks,
        })
    return maps


def kernel(x, Wq, Wk, Wv, Wo):
    x = np.asarray(x, dtype=np.float32)
    Wq = np.asarray(Wq, dtype=np.float32)
    Wk = np.asarray(Wk, dtype=np.float32)
    Wv = np.asarray(Wv, dtype=np.float32)
    Wo = np.asarray(Wo, dtype=np.float32)

    if "nc" not in _cache:
        _cache["nc"] = _build_kernel()
    nc = _cache["nc"]

    res = run_bass_kernel_spmd(nc, _in_maps(x, Wq, Wk, Wv, Wo),
                               list(range(NCORES)))
    out = np.empty((BATCH, SEQ, MODEL_DIM), dtype=np.float32)
    for c in range(NCORES):
        b, g = c // 4, c % 4
        out[b, :, g * QF:(g + 1) * QF] = res.results[c]["out"]
    return out
